# revision 31
# baseline (speedup 1.0000x reference)
"""GQA attention (B=1, L=2048, D=2048, H=32, KV=8, HD=64) + RoPE + causal mask,
tensor-parallel over heads across 8 TRN2 NeuronCores.

Core i owns KV head i and Q heads 4i..4i+3. Each core computes
partial_i = O_i @ wo_i; the host sums the 8 bf16 partials.

Key design points (see trace-driven history in the session notes):
- Host pre-casts x/weights/trig/mask to bf16: no on-device casts, half DMA.
- RoPE via target-aligned weight permutation: 2 full-128-lane DVE products
  (cos-aligned, sign-folded sin) + DMA partition swap + 1 full-lane add.
- S matmuls row-tiled per head pair (rows 0:64 / 64:128) for PE subarray
  concurrency; kt/vn stationary operands shared within an iteration.
- softmax denominator via ones-column in the V stationary; reciprocal on 32
  lanes after an SBUF->SBUF DMA reshape [1,1024]->[32,32].
- A@V matmuls are emitted with a 2-iteration lag behind the S/exp chain so
  the PE never dispatch-stalls on exp completion.
- Q projections for the second query half run at the j0->j1 attention
  boundary through the S-psum ring (the head is x-DMA-bound; this puts the
  first S ~20us earlier).
- O@wo tiles ride the S-psum ring interleaved into the j1 sweeps; the tail
  drains through a 4-deep psum rotation with evacuations on ACT+DVE.
"""

import numpy as np
import ml_dtypes

try:
    import concourse  # noqa: F401
except ImportError:
    import sys as _sys
    for _p in ("/opt/trn_rl_repo", "/root/.axon_site/_ro/trn_rl_repo"):
        if _p not in _sys.path:
            _sys.path.insert(0, _p)

B, L, D = 1, 2048, 2048
H, KV, HD = 32, 8, 64
NCORES = 8
P = 128
KT = D // P          # 16 contraction tiles
LB = L // P          # 16 key blocks

BF16 = ml_dtypes.bfloat16


def _build_nc(reps: int = 1):
    import concourse.bass as bass
    import concourse.mybir as mybir
    import concourse.tile as tile
    from concourse import bacc
    from concourse.bass import ts, ds
    from concourse.masks import make_identity

    f32 = mybir.dt.float32
    bf16 = mybir.dt.bfloat16
    EXP = mybir.ActivationFunctionType.Exp
    ADD = mybir.AluOpType.add
    MULT = mybir.AluOpType.mult

    nc = bacc.Bacc(None, target_bir_lowering=False, debug=False)

    x_t = nc.declare_dram_parameter("x_t", [D, L], bf16, isOutput=False)
    wq_p = nc.declare_dram_parameter("wq_p", [D, 256], bf16, isOutput=False)
    wkv = nc.declare_dram_parameter("wkv", [D, 128], bf16, isOutput=False)
    wo_p = nc.declare_dram_parameter("wo_p", [256, D], bf16, isOutput=False)
    cos64 = nc.declare_dram_parameter("cos64", [64, L], f32, isOutput=False)
    sin64 = nc.declare_dram_parameter("sin64", [64, L], f32, isOutput=False)
    mask_d = nc.declare_dram_parameter("mask_d", [P, LB * P], bf16, isOutput=False)
    part = nc.declare_dram_parameter("part", [L, D], bf16, isOutput=True)

    with tile.TileContext(nc) as tc:
        with tc.tile_pool(name="persist", bufs=1) as pp:
          for _rep in range(reps):
            # ---------- persistent SBUF tensors ----------
            qt_sb = pp.tile([P, 2, L], bf16, tag="qt")     # [Ar|Ai|Br|Bi, pb, q]
            kt_sb = pp.tile([P, L], bf16, tag="kt")        # [Kr|Ki, k]; dup 64:128
            vn_sb = pp.tile([P, LB, 65], bf16, tag="vn")   # [k%128, kb, dd|1]
            ot_sb = pp.tile([P, 2, L], bf16, tag="ot")     # [64*hh+dd, pb, q]
            wq_sb = pp.tile([P, KT, 256], bf16, tag="wq_b")
            kv_sb = pp.tile([P, KT, 128], bf16, tag="kv_b")
            wo_sb = pp.tile([P, 2, L], bf16, tag="wo_b")   # [hc%128, t, d]
            cosb = pp.tile([P, L], f32, tag="cosb")        # [c;c;c;c] rows
            sinb = pp.tile([P, L], f32, tag="sinb")        # [+s;-s;+s;-s]
            em_sb = pp.tile([P, LB * P], bf16, tag="em")   # exp(mask/8) diag factor
            vt_tmp = pp.tile([64, L], bf16, tag="vt")
            ident = pp.tile([64, 64], bf16, tag="ident")

            make_identity(nc, ident[:])
            nc.vector.memset(vn_sb[:, :, 64:65], 1.0)

            with tc.tile_pool(name="xt", bufs=1) as xtp:
              xt_sb = xtp.tile([P, KT, L], bf16, tag="xt_b")

              with (
                tc.tile_pool(name="stage", bufs=1) as stg,
                tc.tile_pool(name="rope", bufs=2) as rtp,
                tc.tile_pool(name="psum_kv", bufs=2, space="PSUM") as pkv,
                tc.tile_pool(name="psum_q", bufs=2, space="PSUM") as pq,
              ):
                mask_sb = stg.tile([P, LB * P], bf16, tag="mask_b")

                # ---- DMA loads: x striped over three DGE paths (sync/scalar
                # HWDGE rings + gpsimd SWDGE); small weights front-loaded ----
                nc.scalar.dma_start(kv_sb[:], wkv.ap().rearrange(
                    "(t p) c -> p t c", p=P))
                nc.scalar.dma_start(cosb[0:64, :], cos64[:, :])
                nc.scalar.dma_start(sinb[0:64, :], sin64[:, :])
                nc.gpsimd.dma_start(mask_sb[:], mask_d[:, :])
                nc.gpsimd.dma_start(wq_sb[:], wq_p.ap().rearrange(
                    "(t p) c -> p t c", p=P))
                x_eng = [nc.sync, nc.scalar, nc.gpsimd]
                for t in range(KT):
                    x_eng[t % 3].dma_start(xt_sb[:, t, :], x_t[ts(t, P), :])
                # duplicate trig rows 0:64 -> 64:128 (Q rope needs 128 rows)
                nc.scalar.dma_start(cosb[64:128, :], cosb[0:64, :])
                nc.scalar.dma_start(sinb[64:128, :], sinb[0:64, :])
                nc.sync.dma_start(wo_sb[:], wo_p.ap().rearrange(
                    "(t p) d -> p t d", p=P))

                # em = exp(mask/8); also warms the ACT exp table set early
                nc.scalar.activation(em_sb[:], mask_sb[:], EXP, scale=0.125)

                def q_rope(pb, j, qps, pool=None):
                    pool = pool if pool is not None else rtp
                    sl = ds(j * 1024, 1024)
                    qa = pool.tile([P, 1024], bf16, tag="qta")
                    qb = pool.tile([P, 1024], bf16, tag="qtb")
                    qbs = pool.tile([P, 1024], bf16, tag="qtbs")
                    nc.vector.tensor_tensor(qa[:], qps[:], cosb[:, sl], MULT)
                    nc.vector.tensor_tensor(qb[:], qps[:], sinb[:, sl], MULT)
                    for g in range(4):
                        src = (g ^ 1) * 32
                        nc.scalar.dma_start(
                            qbs[ds(g * 32, 32), :], qb[ds(src, 32), :])
                    nc.vector.tensor_tensor(qt_sb[:, pb, sl], qa[:], qbs[:], ADD)

                # ---- K|V and Q(j=0) projections chase the x stream ----
                kvps0 = pkv.tile([P, 1024], f32, tag="kv_ps")
                kvps1 = pkv.tile([P, 1024], f32, tag="kv_ps")
                qps00 = pq.tile([P, 1024], f32, tag="q_ps")
                qps10 = pq.tile([P, 1024], f32, tag="q_ps")
                for t in range(KT):
                    for hf in range(2):
                        nc.tensor.matmul(
                            kvps0[:, ts(hf, 512)], kv_sb[:, t, :],
                            xt_sb[:, t, ds(hf * 512, 512)],
                            start=(t == 0), stop=(t == KT - 1))
                        nc.tensor.matmul(
                            kvps1[:, ts(hf, 512)], kv_sb[:, t, :],
                            xt_sb[:, t, ds(1024 + hf * 512, 512)],
                            start=(t == 0), stop=(t == KT - 1))
                        nc.tensor.matmul(
                            qps00[:, ts(hf, 512)], wq_sb[:, t, ds(0, P)],
                            xt_sb[:, t, ds(hf * 512, 512)],
                            start=(t == 0), stop=(t == KT - 1))
                        nc.tensor.matmul(
                            qps10[:, ts(hf, 512)], wq_sb[:, t, ds(P, P)],
                            xt_sb[:, t, ds(hf * 512, 512)],
                            start=(t == 0), stop=(t == KT - 1))

                # V^T -> bf16 staging first (ACT), so the PE transposes can
                # run concurrently with the DVE rope work below
                nc.scalar.copy(vt_tmp[:, 0:1024], kvps0[64:128, :])
                nc.scalar.copy(vt_tmp[:, 1024:2048], kvps1[64:128, :])

                # K rope per j2-half (DVE; overlaps the PE transposes)
                for j2, kvps in ((0, kvps0), (1, kvps1)):
                    sl = ds(j2 * 1024, 1024)
                    ta = rtp.tile([64, 1024], bf16, tag="kta")
                    tb = rtp.tile([64, 1024], bf16, tag="ktb")
                    tbs = rtp.tile([64, 1024], bf16, tag="ktbs")
                    nc.vector.tensor_tensor(ta[:], kvps[0:64, :], cosb[0:64, sl], MULT)
                    nc.vector.tensor_tensor(tb[:], kvps[0:64, :], sinb[0:64, sl], MULT)
                    nc.scalar.dma_start(tbs[0:32, :], tb[32:64, :])
                    nc.scalar.dma_start(tbs[32:64, :], tb[0:32, :])
                    nc.vector.tensor_tensor(kt_sb[0:64, sl], ta[:], tbs[:], ADD)

                q_rope(0, 0, qps00)
                q_rope(1, 0, qps10)

                # kt duplicate into partitions 64:128 (for row-tiled S)
                nc.sync.dma_start(kt_sb[64:128, :], kt_sb[0:64, :])

                # V^T -> V natural via PE transpose (rides the q_ps psum
                # ring; runs concurrently with the DVE rope tail above)
                for kb in range(LB):
                    vps = pq.tile([P, 64], bf16, tag="q_ps")
                    nc.tensor.matmul(
                        vps[:], vt_tmp[:, ts(kb, P)], ident[:],
                        start=True, stop=True, is_transpose=True,
                    )
                    nc.vector.tensor_copy(vn_sb[:, kb, 0:64], vps[:])

                # ---- Q projections for j=1 (xt already resident) ----
                for pb in range(2):
                    qps = pq.tile([P, 1024], f32, tag="q_ps")
                    for t in range(KT):
                        for hf in range(2):
                            nc.tensor.matmul(
                                qps[:, ts(hf, 512)],
                                wq_sb[:, t, ds(pb * P, P)],
                                xt_sb[:, t, ds(1024 + hf * 512, 512)],
                                start=(t == 0), stop=(t == KT - 1),
                            )
                    q_rope(pb, 1, qps)

              # ---------- attention + O @ wo ----------
              with (
                tc.tile_pool(name="attn_sb", bufs=10) as asb,
                tc.tile_pool(name="norm_sb", bufs=3) as nsb,
                tc.tile_pool(name="out_sb", bufs=4) as osb,
                tc.tile_pool(name="psum_s", bufs=2, space="PSUM") as pss,
                tc.tile_pool(name="psum_acc", bufs=2, space="PSUM") as pacc,
              ):
                def emit_wo_tile(lq, n2, evac_scalar=False, pool=None):
                    out_eng = nc.sync if (lq + n2) % 2 == 0 else nc.gpsimd
                    if pool is None:
                        pool = pss
                    wps = pool.tile([P, 1024], f32,
                                    tag="s_ps" if pool is pss else "acc")
                    for t in range(2):
                        for hf2 in range(2):
                            nc.tensor.matmul(
                                wps[:, ts(hf2, 512)],
                                ot_sb[:, t, ts(lq, P)],
                                wo_sb[:, t, ds(n2 * 1024 + hf2 * 512, 512)],
                                start=(t == 0), stop=(t == 1),
                            )
                    ob = osb.tile([P, 1024], bf16, tag="o_sb")
                    if evac_scalar:
                        nc.scalar.copy(ob[:], wps[:])
                    else:
                        nc.vector.tensor_copy(ob[:], wps[:])
                    out_eng.dma_start(
                        part[ts(lq, P), ds(n2 * 1024, 1024)], ob[:])

                wo_pending = []
                for j in range(2):
                    kb_hi = 8 * j + 7
                    for pair in range(2):
                        avA = pacc.tile([65, 1024], f32, tag="acc")
                        avB = pacc.tile([65, 1024], f32, tag="acc")
                        av_queue = []

                        def flush_av():
                            kb2, etA2, etB2, c0a2, c0b2, a_on2 = av_queue.pop(0)
                            lhs_v = vn_sb[:, kb2, :]
                            for (av, et) in ((avA, etA2), (avB, etB2)):
                                if a_on2:
                                    nc.tensor.matmul(
                                        av[:, ds(c0a2, 512 - c0a2)], lhs_v,
                                        et[:, ds(c0a2, 512 - c0a2)],
                                        start=(kb2 == 0),
                                        stop=(kb2 == 8 * j + 3),
                                    )
                                nc.tensor.matmul(
                                    av[:, ds(512 + c0b2, 512 - c0b2)], lhs_v,
                                    et[:, ds(512 + c0b2, 512 - c0b2)],
                                    start=(kb2 == 0), stop=(kb2 == kb_hi),
                                )

                        for kb in range(kb_hi + 1):
                            c0a = max(0, kb - 8 * j) * P
                            c0b = max(0, kb - (8 * j + 4)) * P
                            a_on = kb <= 8 * j + 3
                            spsA = pss.tile([P, 1024], f32, tag="s_ps")
                            spsB = pss.tile([P, 1024], f32, tag="s_ps")
                            for (hh, sps) in ((0, spsA), (1, spsB)):
                                lhs_k = kt_sb[ds(64 * hh, 64), ts(kb, P)]
                                if a_on:
                                    nc.tensor.matmul(
                                        sps[:, ds(c0a, 512 - c0a)], lhs_k,
                                        qt_sb[ds(64 * hh, 64), pair,
                                              ds(2 * j * 512 + c0a, 512 - c0a)],
                                        start=True, stop=True,
                                    )
                                nc.tensor.matmul(
                                    sps[:, ds(512 + c0b, 512 - c0b)], lhs_k,
                                    qt_sb[ds(64 * hh, 64), pair,
                                          ds((2 * j + 1) * 512 + c0b, 512 - c0b)],
                                    start=True, stop=True,
                                )
                            e0 = c0a if a_on else 512 + c0b
                            etA = asb.tile([P, 1024], bf16, tag="e_t")
                            etB = asb.tile([P, 1024], bf16, tag="e_t")
                            for (sps, et) in ((spsA, etA), (spsB, etB)):
                                nc.scalar.activation(
                                    et[:, ds(e0, 1024 - e0)],
                                    sps[:, ds(e0, 1024 - e0)], EXP, scale=0.125,
                                )
                                if a_on and kb >= 8 * j:
                                    nc.vector.tensor_tensor(
                                        et[:, ds(c0a, P)], et[:, ds(c0a, P)],
                                        em_sb[:, ts(kb, P)], MULT,
                                    )
                                if kb >= 8 * j + 4:
                                    nc.vector.tensor_tensor(
                                        et[:, ds(512 + c0b, P)],
                                        et[:, ds(512 + c0b, P)],
                                        em_sb[:, ts(kb, P)], MULT,
                                    )
                            # pad the PE with j0's O@wo work while the exps
                            # run (after the S matmuls so the S->exp chain is
                            # never head-blocked in the PE queue)
                            if wo_pending and (kb % 2 == 1) and (
                                    pair == 1 or kb >= 5):
                                emit_wo_tile(*wo_pending.pop(0))
                            av_queue.append((kb, etA, etB, c0a, c0b, a_on))
                            flush_av()
                        while av_queue:
                            flush_av()
                        # normalize the two heads of this pair
                        for (hh, av) in ((0, avA), (1, avB)):
                            oev = nsb.tile([65, 1024], bf16, tag="o_ev")
                            nc.vector.tensor_copy(oev[:], av[:])
                            rc = nsb.tile([32, 32], bf16, tag="r_c")
                            nc.gpsimd.dma_start(rc[:], oev[64:65, :])
                            ri = nsb.tile([32, 32], bf16, tag="r_i")
                            with nc.allow_low_precision(
                                    reason="softmax denom; 2e-2 tolerance"):
                                nc.vector.reciprocal(ri[:], rc[:])
                            rr = nsb.tile([1, 1024], bf16, tag="r_r")
                            nc.gpsimd.dma_start(rr[:], ri[:])
                            rb = nsb.tile([64, 1024], bf16, tag="r_b")
                            nc.gpsimd.partition_broadcast(rb[:], rr[:])
                            nc.vector.tensor_tensor(
                                ot_sb[ds(64 * hh, 64), pair, ds(j * 1024, 1024)],
                                oev[0:64, :], rb[:], MULT,
                            )

                    # queue this j's O@wo tiles; j0's interleave into the j1
                    # sweeps (via the shared s_ps psum ring), j1's drain below
                    wo_pending += [(lq, n2)
                                   for lq in range(8 * j, 8 * j + 8)
                                   for n2 in range(2)]

                # drain remaining O@wo work with a 4-deep psum rotation
                # (both pools); alternate the psum evacuation between ACT
                # (idle at the tail) and DVE
                for i, (lq, n2) in enumerate(wo_pending):
                    emit_wo_tile(lq, n2, evac_scalar=(i % 2 == 0),
                                 pool=(pss if i % 2 == 0 else pacc))

    nc.compile()
    return nc


_NC_CACHE = None


def _get_nc():
    global _NC_CACHE
    if _NC_CACHE is None:
        _NC_CACHE = _build_nc()
    return _NC_CACHE


def _shard_inputs(x, wq, wk, wv, wo, freqs_cos, freqs_sin, mask):
    """Host-side shard prep: layout transforms + dtype pre-casts."""
    f = np.float32
    # de-interleave (even, odd) feature pairs within a 64-wide head
    perm = np.empty(64, np.int64)
    perm[:32] = 2 * np.arange(32)
    perm[32:] = 2 * np.arange(32) + 1

    x_t = np.ascontiguousarray(np.asarray(x, f).reshape(L, D).T).astype(BF16)
    cosT = np.ascontiguousarray(np.asarray(freqs_cos, f).T)   # [32, L]
    sinT = np.ascontiguousarray(np.asarray(freqs_sin, f).T)
    cos64 = np.ascontiguousarray(np.concatenate([cosT, cosT], 0)).astype(f)
    sin64 = np.ascontiguousarray(np.concatenate([sinT, -sinT], 0)).astype(f)
    mask = np.asarray(mask, f)
    # mask_d[k, kb*128 + q] = mask[kb*128+q, kb*128+k]  (transposed diag blocks)
    md = np.empty((P, LB * P), f)
    for b in range(LB):
        md[:, b * P:(b + 1) * P] = mask[b * P:(b + 1) * P, b * P:(b + 1) * P].T
    md = np.ascontiguousarray(md).astype(BF16)

    wq = np.asarray(wq, f)
    wk = np.asarray(wk, f)
    wv = np.asarray(wv, f)
    wo = np.asarray(wo, f)

    in_maps = []
    for i in range(NCORES):
        wq_i = wq[:, 4 * i * 64:(4 * i + 4) * 64]
        cols = []
        for pb in range(2):
            A = wq_i[:, (2 * pb) * 64:(2 * pb + 1) * 64][:, perm]
            Bc = wq_i[:, (2 * pb + 1) * 64:(2 * pb + 2) * 64][:, perm]
            cols.append(np.concatenate([A, Bc], 1))
        wq_pm = np.ascontiguousarray(np.concatenate(cols, 1)).astype(BF16)
        wk_p = wk[:, i * 64:(i + 1) * 64][:, perm]
        wv_i = wv[:, i * 64:(i + 1) * 64]
        wkv_m = np.ascontiguousarray(
            np.concatenate([wk_p, wv_i], 1)).astype(BF16)
        wo_i = np.ascontiguousarray(
            wo[4 * i * 64:(4 * i + 4) * 64, :]).astype(BF16)
        in_maps.append({
            "x_t": x_t, "wq_p": wq_pm, "wkv": wkv_m, "wo_p": wo_i,
            "cos64": cos64, "sin64": sin64, "mask_d": md,
        })
    return in_maps


_last_results = None


def kernel(x, wq, wk, wv, wo, freqs_cos, freqs_sin, mask):
    global _last_results
    from concourse.bass_utils import run_bass_kernel_spmd

    nc = _get_nc()
    in_maps = _shard_inputs(x, wq, wk, wv, wo, freqs_cos, freqs_sin, mask)
    res = run_bass_kernel_spmd(nc, in_maps, core_ids=list(range(NCORES)))
    _last_results = res
    out = np.zeros((L, D), np.float32)
    for i in range(NCORES):
        out += np.asarray(res.results[i]["part"]).astype(np.float32)
    return out.reshape(B, L, D)


# revision 32
# speedup vs baseline: 1.0347x; 1.0347x over previous
"""GQA attention (B=1, L=2048, D=2048, H=32, KV=8, HD=64) + RoPE + causal mask,
tensor-parallel over heads across 8 TRN2 NeuronCores.

Core i owns KV head i and Q heads 4i..4i+3. Each core computes
partial_i = O_i @ wo_i; the host sums the 8 bf16 partials.

Key design points (see trace-driven history in the session notes):
- Host pre-casts x/weights/trig/mask to bf16: no on-device casts, half DMA.
- RoPE via target-aligned weight permutation: 2 full-128-lane DVE products
  (cos-aligned, sign-folded sin) + DMA partition swap + 1 full-lane add.
- S matmuls row-tiled per head pair (rows 0:64 / 64:128) for PE subarray
  concurrency; kt/vn stationary operands shared within an iteration.
- softmax denominator via ones-column in the V stationary; reciprocal on 32
  lanes after an SBUF->SBUF DMA reshape [1,1024]->[32,32].
- A@V matmuls are emitted with a 2-iteration lag behind the S/exp chain so
  the PE never dispatch-stalls on exp completion.
- Q projections for the second query half run at the j0->j1 attention
  boundary through the S-psum ring (the head is x-DMA-bound; this puts the
  first S ~20us earlier).
- O@wo tiles ride the S-psum ring interleaved into the j1 sweeps; the tail
  drains through a 4-deep psum rotation with evacuations on ACT+DVE.
"""

import numpy as np
import ml_dtypes

try:
    import concourse  # noqa: F401
except ImportError:
    import sys as _sys
    for _p in ("/opt/trn_rl_repo", "/root/.axon_site/_ro/trn_rl_repo"):
        if _p not in _sys.path:
            _sys.path.insert(0, _p)

B, L, D = 1, 2048, 2048
H, KV, HD = 32, 8, 64
NCORES = 8
P = 128
KT = D // P          # 16 contraction tiles
LB = L // P          # 16 key blocks

BF16 = ml_dtypes.bfloat16


def _build_nc(reps: int = 1):
    import concourse.bass as bass
    import concourse.mybir as mybir
    import concourse.tile as tile
    from concourse import bacc
    from concourse.bass import ts, ds
    from concourse.masks import make_identity

    f32 = mybir.dt.float32
    bf16 = mybir.dt.bfloat16
    EXP = mybir.ActivationFunctionType.Exp
    ADD = mybir.AluOpType.add
    MULT = mybir.AluOpType.mult

    nc = bacc.Bacc(None, target_bir_lowering=False, debug=False)

    x_t = nc.declare_dram_parameter("x_t", [D, L], bf16, isOutput=False)
    wq_p = nc.declare_dram_parameter("wq_p", [D, 256], bf16, isOutput=False)
    wkv = nc.declare_dram_parameter("wkv", [D, 128], bf16, isOutput=False)
    wo_p = nc.declare_dram_parameter("wo_p", [256, D], bf16, isOutput=False)
    cos64 = nc.declare_dram_parameter("cos64", [64, L], f32, isOutput=False)
    sin64 = nc.declare_dram_parameter("sin64", [64, L], f32, isOutput=False)
    mask_d = nc.declare_dram_parameter("mask_d", [P, LB * P], bf16, isOutput=False)
    part = nc.declare_dram_parameter("part", [L, D], bf16, isOutput=True)

    with tile.TileContext(nc) as tc:
        with tc.tile_pool(name="persist", bufs=1) as pp:
          for _rep in range(reps):
            # ---------- persistent SBUF tensors ----------
            qt_sb = pp.tile([P, 2, L], bf16, tag="qt")     # [Ar|Ai|Br|Bi, pb, q]
            kt_sb = pp.tile([P, L], bf16, tag="kt")        # [Kr|Ki, k]; dup 64:128
            vn_sb = pp.tile([P, LB, 65], bf16, tag="vn")   # [k%128, kb, dd|1]
            ot_sb = pp.tile([P, 2, L], bf16, tag="ot")     # [64*hh+dd, pb, q]
            wq_sb = pp.tile([P, KT, 256], bf16, tag="wq_b")
            kv_sb = pp.tile([P, KT, 128], bf16, tag="kv_b")
            wo_sb = pp.tile([P, 2, L], bf16, tag="wo_b")   # [hc%128, t, d]
            cosb = pp.tile([P, L], f32, tag="cosb")        # [c;c;c;c] rows
            sinb = pp.tile([P, L], f32, tag="sinb")        # [+s;-s;+s;-s]
            em_sb = pp.tile([P, LB * P], bf16, tag="em")   # exp(mask/8) diag factor
            vt_tmp = pp.tile([64, L], bf16, tag="vt")
            ident = pp.tile([64, 64], bf16, tag="ident")

            make_identity(nc, ident[:])
            nc.vector.memset(vn_sb[:, :, 64:65], 1.0)

            with tc.tile_pool(name="xt", bufs=1) as xtp:
              xt_sb = xtp.tile([P, KT, L], bf16, tag="xt_b")

              with (
                tc.tile_pool(name="stage", bufs=1) as stg,
                tc.tile_pool(name="rope", bufs=2) as rtp,
                tc.tile_pool(name="psum_kv", bufs=2, space="PSUM") as pkv,
                tc.tile_pool(name="psum_q", bufs=2, space="PSUM") as pq,
              ):
                mask_sb = stg.tile([P, LB * P], bf16, tag="mask_b")

                # ---- DMA loads: x striped over three DGE paths (sync/scalar
                # HWDGE rings + gpsimd SWDGE); small weights front-loaded ----
                nc.scalar.dma_start(kv_sb[:], wkv.ap().rearrange(
                    "(t p) c -> p t c", p=P))
                nc.scalar.dma_start(cosb[0:64, :], cos64[:, :])
                nc.scalar.dma_start(sinb[0:64, :], sin64[:, :])
                nc.gpsimd.dma_start(mask_sb[:], mask_d[:, :])
                nc.gpsimd.dma_start(wq_sb[:], wq_p.ap().rearrange(
                    "(t p) c -> p t c", p=P))
                x_eng = [nc.sync, nc.scalar, nc.gpsimd]
                for t in range(KT):
                    x_eng[t % 3].dma_start(xt_sb[:, t, :], x_t[ts(t, P), :])
                # duplicate trig rows 0:64 -> 64:128 (Q rope needs 128 rows)
                nc.scalar.dma_start(cosb[64:128, :], cosb[0:64, :])
                nc.scalar.dma_start(sinb[64:128, :], sinb[0:64, :])
                nc.sync.dma_start(wo_sb[:], wo_p.ap().rearrange(
                    "(t p) d -> p t d", p=P))

                # em = exp(mask/8); also warms the ACT exp table set early
                nc.scalar.activation(em_sb[:], mask_sb[:], EXP, scale=0.125)

                def q_rope(pb, j, qps, pool=None):
                    pool = pool if pool is not None else rtp
                    sl = ds(j * 1024, 1024)
                    qa = pool.tile([P, 1024], bf16, tag="qta")
                    qb = pool.tile([P, 1024], bf16, tag="qtb")
                    qbs = pool.tile([P, 1024], bf16, tag="qtbs")
                    nc.vector.tensor_tensor(qa[:], qps[:], cosb[:, sl], MULT)
                    nc.vector.tensor_tensor(qb[:], qps[:], sinb[:, sl], MULT)
                    for g in range(4):
                        src = (g ^ 1) * 32
                        nc.scalar.dma_start(
                            qbs[ds(g * 32, 32), :], qb[ds(src, 32), :])
                    nc.vector.tensor_tensor(qt_sb[:, pb, sl], qa[:], qbs[:], ADD)

                # ---- K|V and Q(j=0) projections chase the x stream ----
                kvps0 = pkv.tile([P, 1024], f32, tag="kv_ps")
                kvps1 = pkv.tile([P, 1024], f32, tag="kv_ps")
                qps00 = pq.tile([P, 1024], f32, tag="q_ps")
                qps10 = pq.tile([P, 1024], f32, tag="q_ps")
                for t in range(KT):
                    for hf in range(2):
                        nc.tensor.matmul(
                            kvps0[:, ts(hf, 512)], kv_sb[:, t, :],
                            xt_sb[:, t, ds(hf * 512, 512)],
                            start=(t == 0), stop=(t == KT - 1))
                        nc.tensor.matmul(
                            kvps1[:, ts(hf, 512)], kv_sb[:, t, :],
                            xt_sb[:, t, ds(1024 + hf * 512, 512)],
                            start=(t == 0), stop=(t == KT - 1))
                        nc.tensor.matmul(
                            qps00[:, ts(hf, 512)], wq_sb[:, t, ds(0, P)],
                            xt_sb[:, t, ds(hf * 512, 512)],
                            start=(t == 0), stop=(t == KT - 1))
                        nc.tensor.matmul(
                            qps10[:, ts(hf, 512)], wq_sb[:, t, ds(P, P)],
                            xt_sb[:, t, ds(hf * 512, 512)],
                            start=(t == 0), stop=(t == KT - 1))

                # V^T -> bf16 staging first (ACT), so the PE transposes can
                # run concurrently with the DVE rope work below
                nc.scalar.copy(vt_tmp[:, 0:1024], kvps0[64:128, :])
                nc.scalar.copy(vt_tmp[:, 1024:2048], kvps1[64:128, :])

                # K rope per j2-half (DVE; overlaps the PE transposes)
                for j2, kvps in ((0, kvps0), (1, kvps1)):
                    sl = ds(j2 * 1024, 1024)
                    ta = rtp.tile([64, 1024], bf16, tag="kta")
                    tb = rtp.tile([64, 1024], bf16, tag="ktb")
                    tbs = rtp.tile([64, 1024], bf16, tag="ktbs")
                    nc.vector.tensor_tensor(ta[:], kvps[0:64, :], cosb[0:64, sl], MULT)
                    nc.vector.tensor_tensor(tb[:], kvps[0:64, :], sinb[0:64, sl], MULT)
                    nc.scalar.dma_start(tbs[0:32, :], tb[32:64, :])
                    nc.scalar.dma_start(tbs[32:64, :], tb[0:32, :])
                    nc.vector.tensor_tensor(kt_sb[0:64, sl], ta[:], tbs[:], ADD)

                q_rope(0, 0, qps00)
                q_rope(1, 0, qps10)

                # kt duplicate into partitions 64:128 (for row-tiled S)
                nc.sync.dma_start(kt_sb[64:128, :], kt_sb[0:64, :])

                # V^T -> V natural via PE transpose (rides the q_ps psum
                # ring; runs concurrently with the DVE rope tail above)
                for kb in range(LB):
                    vps = pq.tile([P, 64], bf16, tag="q_ps")
                    nc.tensor.matmul(
                        vps[:], vt_tmp[:, ts(kb, P)], ident[:],
                        start=True, stop=True, is_transpose=True,
                    )
                    nc.vector.tensor_copy(vn_sb[:, kb, 0:64], vps[:])

                # ---- Q projections for j=1 (xt already resident) ----
                for pb in range(2):
                    qps = pq.tile([P, 1024], f32, tag="q_ps")
                    for t in range(KT):
                        for hf in range(2):
                            nc.tensor.matmul(
                                qps[:, ts(hf, 512)],
                                wq_sb[:, t, ds(pb * P, P)],
                                xt_sb[:, t, ds(1024 + hf * 512, 512)],
                                start=(t == 0), stop=(t == KT - 1),
                            )
                    q_rope(pb, 1, qps)

              # ---------- attention + O @ wo ----------
              with (
                tc.tile_pool(name="attn_sb", bufs=10) as asb,
                tc.tile_pool(name="norm_sb", bufs=3) as nsb,
                tc.tile_pool(name="out_sb", bufs=4) as osb,
                tc.tile_pool(name="psum_s", bufs=2, space="PSUM") as pss,
                tc.tile_pool(name="psum_acc", bufs=2, space="PSUM") as pacc,
              ):
                def emit_wo_tile(lq, n2, evac_scalar=False, pool=None):
                    out_eng = nc.sync if (lq + n2) % 2 == 0 else nc.gpsimd
                    if pool is None:
                        pool = pss
                    wps = pool.tile([P, 1024], f32,
                                    tag="s_ps" if pool is pss else "acc")
                    for t in range(2):
                        for hf2 in range(2):
                            nc.tensor.matmul(
                                wps[:, ts(hf2, 512)],
                                ot_sb[:, t, ts(lq, P)],
                                wo_sb[:, t, ds(n2 * 1024 + hf2 * 512, 512)],
                                start=(t == 0), stop=(t == 1),
                            )
                    ob = osb.tile([P, 1024], bf16, tag="o_sb")
                    if evac_scalar:
                        nc.scalar.copy(ob[:], wps[:])
                    else:
                        nc.vector.tensor_copy(ob[:], wps[:])
                    out_eng.dma_start(
                        part[ts(lq, P), ds(n2 * 1024, 1024)], ob[:])

                wo_pending = []
                for j in range(2):
                    kb_hi = 8 * j + 7
                    for pair in range(2):
                        avA = pacc.tile([65, 1024], f32, tag="acc")
                        avB = pacc.tile([65, 1024], f32, tag="acc")
                        av_queue = []

                        def flush_av():
                            kb2, etA2, etB2, c0a2, c0b2, a_on2 = av_queue.pop(0)
                            lhs_v = vn_sb[:, kb2, :]
                            for (av, et) in ((avA, etA2), (avB, etB2)):
                                if a_on2:
                                    nc.tensor.matmul(
                                        av[:, ds(c0a2, 512 - c0a2)], lhs_v,
                                        et[:, ds(c0a2, 512 - c0a2)],
                                        start=(kb2 == 0),
                                        stop=(kb2 == 8 * j + 3),
                                    )
                                nc.tensor.matmul(
                                    av[:, ds(512 + c0b2, 512 - c0b2)], lhs_v,
                                    et[:, ds(512 + c0b2, 512 - c0b2)],
                                    start=(kb2 == 0), stop=(kb2 == kb_hi),
                                )

                        for kb in range(kb_hi + 1):
                            c0a = max(0, kb - 8 * j) * P
                            c0b = max(0, kb - (8 * j + 4)) * P
                            a_on = kb <= 8 * j + 3
                            spsA = pss.tile([P, 1024], f32, tag="s_ps")
                            spsB = pss.tile([P, 1024], f32, tag="s_ps")
                            for (hh, sps) in ((0, spsA), (1, spsB)):
                                lhs_k = kt_sb[ds(64 * hh, 64), ts(kb, P)]
                                if a_on:
                                    nc.tensor.matmul(
                                        sps[:, ds(c0a, 512 - c0a)], lhs_k,
                                        qt_sb[ds(64 * hh, 64), pair,
                                              ds(2 * j * 512 + c0a, 512 - c0a)],
                                        start=True, stop=True,
                                    )
                                nc.tensor.matmul(
                                    sps[:, ds(512 + c0b, 512 - c0b)], lhs_k,
                                    qt_sb[ds(64 * hh, 64), pair,
                                          ds((2 * j + 1) * 512 + c0b, 512 - c0b)],
                                    start=True, stop=True,
                                )
                            e0 = c0a if a_on else 512 + c0b
                            etA = asb.tile([P, 1024], bf16, tag="e_t")
                            etB = asb.tile([P, 1024], bf16, tag="e_t")
                            for (sps, et) in ((spsA, etA), (spsB, etB)):
                                nc.scalar.activation(
                                    et[:, ds(e0, 1024 - e0)],
                                    sps[:, ds(e0, 1024 - e0)], EXP, scale=0.125,
                                )
                                if a_on and kb >= 8 * j:
                                    nc.vector.tensor_tensor(
                                        et[:, ds(c0a, P)], et[:, ds(c0a, P)],
                                        em_sb[:, ts(kb, P)], MULT,
                                    )
                                if kb >= 8 * j + 4:
                                    nc.vector.tensor_tensor(
                                        et[:, ds(512 + c0b, P)],
                                        et[:, ds(512 + c0b, P)],
                                        em_sb[:, ts(kb, P)], MULT,
                                    )
                            # pad the PE with j0's O@wo work while the exps
                            # run (after the S matmuls so the S->exp chain is
                            # never head-blocked in the PE queue)
                            if wo_pending and (kb % 2 == 1) and (
                                    pair == 1 or kb >= 5):
                                emit_wo_tile(*wo_pending.pop(0))
                            # A@V runs 1 iteration behind S/exp so its
                            # dispatch never waits on an in-flight exp
                            av_queue.append((kb, etA, etB, c0a, c0b, a_on))
                            if len(av_queue) > 1:
                                flush_av()
                        while av_queue:
                            flush_av()
                        # normalize the two heads of this pair
                        for (hh, av) in ((0, avA), (1, avB)):
                            oev = nsb.tile([65, 1024], bf16, tag="o_ev")
                            nc.vector.tensor_copy(oev[:], av[:])
                            rc = nsb.tile([32, 32], bf16, tag="r_c")
                            nc.gpsimd.dma_start(rc[:], oev[64:65, :])
                            ri = nsb.tile([32, 32], bf16, tag="r_i")
                            with nc.allow_low_precision(
                                    reason="softmax denom; 2e-2 tolerance"):
                                nc.vector.reciprocal(ri[:], rc[:])
                            rr = nsb.tile([1, 1024], bf16, tag="r_r")
                            nc.gpsimd.dma_start(rr[:], ri[:])
                            rb = nsb.tile([64, 1024], bf16, tag="r_b")
                            nc.gpsimd.partition_broadcast(rb[:], rr[:])
                            nc.vector.tensor_tensor(
                                ot_sb[ds(64 * hh, 64), pair, ds(j * 1024, 1024)],
                                oev[0:64, :], rb[:], MULT,
                            )

                    # queue this j's O@wo tiles; j0's interleave into the j1
                    # sweeps (via the shared s_ps psum ring), j1's drain below
                    wo_pending += [(lq, n2)
                                   for lq in range(8 * j, 8 * j + 8)
                                   for n2 in range(2)]

                # drain remaining O@wo work with a 4-deep psum rotation
                # (both pools); alternate the psum evacuation between ACT
                # (idle at the tail) and DVE
                for i, (lq, n2) in enumerate(wo_pending):
                    emit_wo_tile(lq, n2, evac_scalar=(i % 2 == 0),
                                 pool=(pss if i % 2 == 0 else pacc))

    nc.compile()
    return nc


_NC_CACHE = None


def _get_nc():
    global _NC_CACHE
    if _NC_CACHE is None:
        _NC_CACHE = _build_nc()
    return _NC_CACHE


def _shard_inputs(x, wq, wk, wv, wo, freqs_cos, freqs_sin, mask):
    """Host-side shard prep: layout transforms + dtype pre-casts."""
    f = np.float32
    # de-interleave (even, odd) feature pairs within a 64-wide head
    perm = np.empty(64, np.int64)
    perm[:32] = 2 * np.arange(32)
    perm[32:] = 2 * np.arange(32) + 1

    x_t = np.ascontiguousarray(np.asarray(x, f).reshape(L, D).T).astype(BF16)
    cosT = np.ascontiguousarray(np.asarray(freqs_cos, f).T)   # [32, L]
    sinT = np.ascontiguousarray(np.asarray(freqs_sin, f).T)
    cos64 = np.ascontiguousarray(np.concatenate([cosT, cosT], 0)).astype(f)
    sin64 = np.ascontiguousarray(np.concatenate([sinT, -sinT], 0)).astype(f)
    mask = np.asarray(mask, f)
    # mask_d[k, kb*128 + q] = mask[kb*128+q, kb*128+k]  (transposed diag blocks)
    md = np.empty((P, LB * P), f)
    for b in range(LB):
        md[:, b * P:(b + 1) * P] = mask[b * P:(b + 1) * P, b * P:(b + 1) * P].T
    md = np.ascontiguousarray(md).astype(BF16)

    wq = np.asarray(wq, f)
    wk = np.asarray(wk, f)
    wv = np.asarray(wv, f)
    wo = np.asarray(wo, f)

    in_maps = []
    for i in range(NCORES):
        wq_i = wq[:, 4 * i * 64:(4 * i + 4) * 64]
        cols = []
        for pb in range(2):
            A = wq_i[:, (2 * pb) * 64:(2 * pb + 1) * 64][:, perm]
            Bc = wq_i[:, (2 * pb + 1) * 64:(2 * pb + 2) * 64][:, perm]
            cols.append(np.concatenate([A, Bc], 1))
        wq_pm = np.ascontiguousarray(np.concatenate(cols, 1)).astype(BF16)
        wk_p = wk[:, i * 64:(i + 1) * 64][:, perm]
        wv_i = wv[:, i * 64:(i + 1) * 64]
        wkv_m = np.ascontiguousarray(
            np.concatenate([wk_p, wv_i], 1)).astype(BF16)
        wo_i = np.ascontiguousarray(
            wo[4 * i * 64:(4 * i + 4) * 64, :]).astype(BF16)
        in_maps.append({
            "x_t": x_t, "wq_p": wq_pm, "wkv": wkv_m, "wo_p": wo_i,
            "cos64": cos64, "sin64": sin64, "mask_d": md,
        })
    return in_maps


_last_results = None


def kernel(x, wq, wk, wv, wo, freqs_cos, freqs_sin, mask):
    global _last_results
    from concourse.bass_utils import run_bass_kernel_spmd

    nc = _get_nc()
    in_maps = _shard_inputs(x, wq, wk, wv, wo, freqs_cos, freqs_sin, mask)
    res = run_bass_kernel_spmd(nc, in_maps, core_ids=list(range(NCORES)))
    _last_results = res
    out = np.zeros((L, D), np.float32)
    for i in range(NCORES):
        out += np.asarray(res.results[i]["part"]).astype(np.float32)
    return out.reshape(B, L, D)
